# revision 17
# baseline (speedup 1.0000x reference)
# GFNet-style block on 8 trn2 NeuronCores, data-parallel over batch.
#
# Per batch element (891 rows x 900 channels):
#   LN1 -> factorized 3D spectral filter -> LN2 -> PE transpose ->
#   low-rank MLP (900->450->1800 gelu ->450->900) -> residual add.
#
# The 3D rfftn/irfftn over the 9x11x9 grid is factorized:
#   S1: rfft along d (9->5c) as a block-diagonal [99,99] matmul per
#       99-row tile (11 hw-groups x 9 rows each), planes j=0..8 =
#       (re0..re4, im1..im4); im0 == 0 for real input.
#   regroup (SBUF->SBUF DMA): rows (g,j) -> [hw, j-block] layout D.
#   S2: full 2D 99-point DFT over (h,w) via dense [99,99] stationaries.
#   cmult: complex weight multiply on DVE directly from PSUM.
#   S3: inverse 2D DFT (im0 output plane dropped: irfft ignores it).
#   regroup back, S4: irfft along d as block-diagonal [99,99] matmul,
#   output lands row-major for LN2.
# This cuts forward+inverse spectral PE work ~2x vs dense 891<->1024
# matmuls.  beta1's filter response (a per-channel constant) rides the
# spectral DC bin of E_re0 scaled by sqrt(891).
#
# Precision: bf16 matmul operands, fp32 PSUM accumulation, fp32 LN stats
# and residual.

import os

if os.environ.get("AXON_H4_ENABLED") == "1":
    _jp = os.environ.get("JAX_PLATFORMS")
    if _jp is not None and "axon" not in _jp:
        os.environ["JAX_PLATFORMS"] = "axon," + _jp

import numpy as np
import ml_dtypes

import concourse.bass as bass
import concourse.tile as tile
from concourse import bacc, mybir
from concourse.bass_utils import run_bass_kernel_spmd
from concourse.masks import make_identity

BQ, H, W, D, DR, C = 64, 9, 11, 9, 5, 900
N = H * W * D            # 891
NCORES = 8
BL = BQ // NCORES        # 8 batch elements per core
R, HID, RA = 450, 1800, 451
EPS = 1e-5
G = 99                   # hw-grid points per d-slice; also tile row count

BF = mybir.dt.bfloat16
F32 = mybir.dt.float32
_BFNP = ml_dtypes.bfloat16


def _chunks(total, size):
    out, o = [], 0
    while o < total:
        out.append((o, min(size, total - o)))
        o += size
    return out


ROW_T = _chunks(N, 128)      # 7 row tiles for L4/residual (last 123)
C_T = _chunks(C, 128)        # 8 channel tiles (last 4)
R_T = _chunks(R, 128)        # 4 (last 66)
RA_T = _chunks(RA, 128)      # 4 (last 67)
HID_T = _chunks(HID, 120)    # 15 x 120
NH = [(0, 446), (446, 445)]  # row halves for matmul free dim
CCH = [(0, 450), (450, 450)]  # channel halves


def _host_constants(inputs):
    """Fold params into the matrices the device kernel consumes."""
    cw = np.asarray(inputs["cw"], np.float64)
    g1 = np.asarray(inputs["gamma1"], np.float64)
    b1 = np.asarray(inputs["beta1"], np.float64)
    g2 = np.asarray(inputs["gamma2"], np.float32)
    b2 = np.asarray(inputs["beta2"], np.float32)
    u1 = np.asarray(inputs["u1_w"], np.float32)
    v1 = np.asarray(inputs["v1_w"], np.float32)
    v1b = np.asarray(inputs["v1_b"], np.float32)
    u2 = np.asarray(inputs["u2_w"], np.float32)
    v2 = np.asarray(inputs["v2_w"], np.float32)
    v2b = np.asarray(inputs["v2_b"], np.float32)

    # ---- spectral stationaries ----
    # d-axis rfft: R9[d, j], j = (re k<=4 | im k=1..4)
    F9r = np.fft.rfft(np.eye(D), norm="ortho", axis=0)       # [k, d]
    R9 = np.zeros((D, D))
    R9[:, 0:5] = F9r.real.T
    R9[:, 5:9] = F9r.imag.T[:, 1:5]
    A1 = np.zeros((G, G))
    # d-axis irfft probes: R9i[j, d]
    R9i = np.zeros((D, D))
    for k in range(5):
        spec = np.zeros(DR, complex); spec[k] = 1.0
        R9i[k] = np.fft.irfft(spec, n=D, norm="ortho")
        if k > 0:
            spec = np.zeros(DR, complex); spec[k] = 1j
            R9i[4 + k] = np.fft.irfft(spec, n=D, norm="ortho")
    A4 = np.zeros((G, G))
    for g in range(11):
        A1[9 * g:9 * g + 9, 9 * g:9 * g + 9] = R9
        A4[9 * g:9 * g + 9, 9 * g:9 * g + 9] = R9i

    # 2D (h,w) DFT: F2[hw, hwf] = FH[hf,h] * FW[wf,w].  Device rows use the
    # relabeled spatial index hw' = 9g + t (t = 99-row tile, g = local
    # group), i.e. hw_orig = 11*(p%9) + p//9, so the regroup DMAs see a
    # single contiguous partition dim on the D/C9 side.
    FH = np.fft.fft(np.eye(H), norm="ortho", axis=0)
    FW = np.fft.fft(np.eye(W), norm="ortho", axis=0)
    F2 = np.kron(FH.T, FW.T)                                  # [hw, hwf]
    perm = np.array([11 * (p % 9) + p // 9 for p in range(G)])
    F2r = F2.real[perm, :].copy()
    F2i = F2.imag[perm, :].copy()

    # complex weight (gamma1 folded), planes re0..4 then im0..4: [99, 10*900]
    wr = (cw[..., 0] * g1).reshape(G, DR, C)
    wi = (cw[..., 1] * g1).reshape(G, DR, C)
    wcat = np.zeros((G, 10 * C), np.float32)
    for k in range(DR):
        wcat[:, k * C:(k + 1) * C] = wr[:, k]
        wcat[:, (5 + k) * C:(6 + k) * C] = wi[:, k]

    # beta1's filter response: constant per channel, rides E_re0 DC bin
    wfull = cw[..., 0] + 1j * cw[..., 1]
    dcs = np.fft.rfftn(np.ones((H, W, D, 1)) * b1[None, None, None, :],
                       axes=(0, 1, 2), norm="ortho")
    off1 = np.fft.irfftn(dcs * wfull, s=(H, W, D), axes=(0, 1, 2),
                         norm="ortho")[0, 0, 0, :]
    off1row = (off1 * np.sqrt(float(N)))[None, :].astype(np.float32)

    # ---- MLP folds (unchanged from dense version) ----
    u1pT = (u1 * g2[None, :]).T.copy()        # [900, 450]
    b1p = np.zeros((128, len(R_T)), np.float32)
    bias1 = u1 @ b2
    for j, (o, sz) in enumerate(R_T):
        b1p[:sz, j] = bias1[o:o + sz]
    v1bp = np.zeros((120, len(HID_T)), np.float32)
    for j, (o, sz) in enumerate(HID_T):
        v1bp[:sz, j] = v1b[o:o + sz]
    v2Ta = np.concatenate([v2.T, v2b[None, :]], axis=0)  # [451, 900]

    bf = lambda a: np.ascontiguousarray(a).astype(_BFNP)
    return {
        "A1": bf(A1), "A4": bf(A4),
        "SFr": bf(F2r), "SFi": bf(F2i), "SFn": bf(-F2i),
        "SGr": bf(F2r.T.copy()), "SGi": bf(F2i.T.copy()),
        "SGn": bf(-F2i.T.copy()),
        "wcat": bf(wcat), "off1row": bf(off1row),
        "u1pT": bf(u1pT), "v1T": bf(v1.T), "u2T": bf(u2.T), "v2Ta": bf(v2Ta),
        "b1p": b1p, "v1bp": v1bp,
        "onesrow": np.ones((1, N), _BFNP),
    }


def build_module(bl=BL, gelu_func=None):
    if gelu_func is None:
        gelu_func = mybir.ActivationFunctionType.Gelu
    nc = bacc.Bacc("TRN2", target_bir_lowering=False, debug=False,
                   enable_asserts=False, num_devices=NCORES)

    x_d = nc.dram_tensor("x", [bl, N, C], F32, kind="ExternalInput").ap()
    out_d = nc.dram_tensor("out", [bl, N, C], F32, kind="ExternalOutput").ap()
    A1_d = nc.dram_tensor("A1", [G, G], BF, kind="ExternalInput").ap()
    A4_d = nc.dram_tensor("A4", [G, G], BF, kind="ExternalInput").ap()
    SFr_d = nc.dram_tensor("SFr", [G, G], BF, kind="ExternalInput").ap()
    SFi_d = nc.dram_tensor("SFi", [G, G], BF, kind="ExternalInput").ap()
    SFn_d = nc.dram_tensor("SFn", [G, G], BF, kind="ExternalInput").ap()
    SGr_d = nc.dram_tensor("SGr", [G, G], BF, kind="ExternalInput").ap()
    SGi_d = nc.dram_tensor("SGi", [G, G], BF, kind="ExternalInput").ap()
    SGn_d = nc.dram_tensor("SGn", [G, G], BF, kind="ExternalInput").ap()
    wcat_d = nc.dram_tensor("wcat", [G, 10 * C], BF, kind="ExternalInput").ap()
    off1_d = nc.dram_tensor("off1row", [1, C], BF, kind="ExternalInput").ap()
    u1pT_d = nc.dram_tensor("u1pT", [C, R], BF, kind="ExternalInput").ap()
    v1T_d = nc.dram_tensor("v1T", [R, HID], BF, kind="ExternalInput").ap()
    u2T_d = nc.dram_tensor("u2T", [HID, R], BF, kind="ExternalInput").ap()
    v2Ta_d = nc.dram_tensor("v2Ta", [RA, C], BF, kind="ExternalInput").ap()
    b1p_d = nc.dram_tensor("b1p", [128, len(R_T)], F32, kind="ExternalInput").ap()
    v1bp_d = nc.dram_tensor("v1bp", [120, len(HID_T)], F32, kind="ExternalInput").ap()
    ones_d = nc.dram_tensor("onesrow", [1, N], BF, kind="ExternalInput").ap()

    MULT = mybir.AluOpType.mult

    with tile.TileContext(nc) as tc:
        with (
            tc.tile_pool(name="const", bufs=1) as const,
            tc.tile_pool(name="xin", bufs=2) as xpool,
            tc.tile_pool(name="stat", bufs=8) as stat,
            tc.tile_pool(name="act", bufs=1) as act,
            tc.tile_pool(name="xres", bufs=2) as xres,
            tc.tile_pool(name="pspec", bufs=4, space="PSUM") as pspec,
            tc.tile_pool(name="psm", bufs=3, space="PSUM") as psm,
        ):
            # ---- persistent constants ----
            def _cload(dram, p, f, tagp):
                t = const.tile([p, f], BF, tag=tagp, name=tagp)
                nc.sync.dma_start(out=t, in_=dram)
                return t

            A1_sb = _cload(A1_d, G, G, "A1")
            SFr_sb = _cload(SFr_d, G, G, "SFr")
            SFi_sb = _cload(SFi_d, G, G, "SFi")
            SFn_sb = _cload(SFn_d, G, G, "SFn")
            wcat_sb = _cload(wcat_d, G, 10 * C, "wc")
            SGr_sb = _cload(SGr_d, G, G, "SGr")
            SGi_sb = _cload(SGi_d, G, G, "SGi")
            SGn_sb = _cload(SGn_d, G, G, "SGn")
            A4_sb = _cload(A4_d, G, G, "A4")
            off1_sb = _cload(off1_d, 1, C, "off1")

            u1pT_sb, v1T_sb, u2T_sb, v2Ta_sb = [], [], [], []
            b1p_sb = const.tile([128, len(R_T)], F32, tag="b1p")
            v1bp_sb = const.tile([120, len(HID_T)], F32, tag="v1bp")

            def _load(pool, dram, parts, cols, tagp):
                tiles = []
                for i, (o, sz) in enumerate(parts):
                    t = pool.tile([sz, cols], BF, tag=f"{tagp}{i}")
                    nc.sync.dma_start(out=t, in_=dram[o:o + sz, :])
                    tiles.append(t)
                return tiles

            def load_mlp_consts():
                # emitted after batch-0's S1 so the startup DMA burst
                # doesn't delay the first matmuls
                u1pT_sb.extend(_load(const, u1pT_d, C_T, R, "u1"))
                v1T_sb.extend(_load(const, v1T_d, R_T, HID, "v1"))
                u2T_sb.extend(_load(const, u2T_d, HID_T, R, "u2"))
                v2Ta_sb.extend(_load(const, v2Ta_d, RA_T, C, "v2"))
                nc.sync.dma_start(out=b1p_sb, in_=b1p_d)
                nc.sync.dma_start(out=v1bp_sb, in_=v1bp_d)

            ident = const.tile([128, 128], BF, tag="ident")
            make_identity(nc, ident)
            epst = const.tile([128, 1], F32, tag="eps")
            nc.vector.memset(epst, EPS)

            def ln_scalars(mv, rs, tag):
                """mv [p,2] (mean, var) -> (scale=rsqrt(var+eps), bias=-mean*scale)"""
                sq = stat.tile([128, 1], F32, tag=f"sq{tag}")
                nc.scalar.activation(sq[:rs], mv[:rs, 1:2],
                                     mybir.ActivationFunctionType.Sqrt,
                                     bias=epst[:rs], scale=1.0)
                rcp = stat.tile([128, 1], F32, tag=f"rc{tag}")
                nc.vector.reciprocal(rcp[:rs], sq[:rs])
                nmu = stat.tile([128, 1], F32, tag=f"nm{tag}")
                nc.vector.scalar_tensor_tensor(
                    out=nmu[:rs], in0=mv[:rs, 0:1], scalar=-1.0, in1=rcp[:rs],
                    op0=MULT, op1=MULT)
                return rcp, nmu

            def ln1_s1_fwd(b):
                """LN1 + d-axis rfft (S1) + regroup DMA -> D [99, 9*900].

                S1 output rows are j-major (11j+g is NOT used; rows stay
                9g+j) collected in one wide tile S1A with the 9 source
                tiles side by side in columns.  The regroup then needs one
                DMA per plane j: src = 11 partitions (stride 9, offset j)
                x (t, c) byte dims; dst = D[:, j-block], whose partition
                index is the relabeled hw' = 9g + t.
                """
                S1A = act.tile([G, 9 * C], BF, tag="E", name="S1A")
                for t in range(9):
                    xc = xpool.tile([G, C], F32, tag="xc")
                    nc.scalar.dma_start(out=xc, in_=x_d[b, G * t:G * t + G, :])
                    st = stat.tile([128, 2, 6], F32, tag="st1")
                    nc.vector.bn_stats(st[:G, 0], xc[:, 0:450])
                    nc.vector.bn_stats(st[:G, 1], xc[:, 450:900])
                    mv = stat.tile([128, 2], F32, tag="mv1")
                    nc.vector.bn_aggr(mv[:G], st[:G])
                    rcp, nmu = ln_scalars(mv, G, "1")
                    s_t = act.tile([G, C], BF, tag="s", bufs=2)
                    nc.scalar.activation(s_t, xc,
                                         mybir.ActivationFunctionType.Identity,
                                         bias=nmu[:G], scale=rcp[:G])
                    for co, cs in CCH:
                        ps = pspec.tile([G, 450], F32, tag="sp", name="ps_s1")
                        nc.tensor.matmul(ps, A1_sb, s_t[:, co:co + cs],
                                         start=True, stop=True)
                        nc.scalar.activation(S1A[:, t * C + co:t * C + co + cs],
                                             ps,
                                             mybir.ActivationFunctionType.Copy)
                Dt = act.tile([G, 9 * C], BF, tag="D", name="Dt")
                sv = S1A.rearrange("(g j) (t c) -> g j t c", g=11, t=9)
                for j in range(9):
                    nc.sync.dma_start(out=Dt[:, j * C:(j + 1) * C], in_=sv[:, j])
                return Dt

            def s2_cmult(b, Dt):
                """2D DFT over hw + complex weight multiply -> E [99, 10*900]."""
                Et = act.tile([G, 10 * C], BF, tag="E", name="Et")
                for co, cs in CCH:
                    for k in range(5):
                        dre = Dt[:, k * C + co:k * C + co + cs]
                        psR = pspec.tile([G, 450], F32, tag="sp", name="psR")
                        psI = pspec.tile([G, 450], F32, tag="sp", name="psI")
                        if k == 0:
                            nc.tensor.matmul(psR, SFr_sb, dre, start=True, stop=True)
                            nc.tensor.matmul(psI, SFi_sb, dre, start=True, stop=True)
                        else:
                            dim_ = Dt[:, (4 + k) * C + co:(4 + k) * C + co + cs]
                            nc.tensor.matmul(psR, SFr_sb, dre, start=True, stop=False)
                            nc.tensor.matmul(psR, SFn_sb, dim_, start=False, stop=True)
                            nc.tensor.matmul(psI, SFi_sb, dre, start=True, stop=False)
                            nc.tensor.matmul(psI, SFr_sb, dim_, start=False, stop=True)
                        wrk = wcat_sb[:, k * C + co:k * C + co + cs]
                        wik = wcat_sb[:, (5 + k) * C + co:(5 + k) * C + co + cs]
                        eRe = Et[:, k * C + co:k * C + co + cs]
                        eIm = Et[:, (5 + k) * C + co:(5 + k) * C + co + cs]
                        u = stat.tile([G, 450], BF, tag="cmu", bufs=2)
                        nc.vector.tensor_mul(u, psI, wik)        # wi*Im
                        nc.vector.tensor_mul(eRe, psR, wrk)      # wr*Re
                        nc.vector.tensor_sub(eRe, eRe, u)
                        u2 = stat.tile([G, 450], BF, tag="cmu2", bufs=2)
                        nc.vector.tensor_mul(u2, psR, wik)       # wi*Re
                        nc.vector.tensor_mul(eIm, psI, wrk)      # wr*Im
                        nc.vector.tensor_add(eIm, eIm, u2)
                        if k == 0:
                            nc.vector.tensor_add(Et[0:1, co:co + cs],
                                                 Et[0:1, co:co + cs],
                                                 off1_sb[0:1, co:co + cs])
                return Et

            def s3_regroup(b, Et):
                """Inverse 2D DFT (drop im0 out-plane) + regroup -> V tiles."""
                # shares the D buffer: D is fully consumed by S2 before
                # S3 writes C9 (WAR tracked by tile framework)
                C9 = act.tile([G, 9 * C], BF, tag="D", name="C9")
                for co, cs in CCH:
                    for j in range(9):
                        k = j if j < 5 else j - 4
                        eRe = Et[:, k * C + co:k * C + co + cs]
                        eIm = Et[:, (5 + k) * C + co:(5 + k) * C + co + cs]
                        ps = pspec.tile([G, 450], F32, tag="sp", name="psC")
                        if j < 5:
                            nc.tensor.matmul(ps, SGr_sb, eRe, start=True, stop=False)
                            nc.tensor.matmul(ps, SGi_sb, eIm, start=False, stop=True)
                        else:
                            nc.tensor.matmul(ps, SGn_sb, eRe, start=True, stop=False)
                            nc.tensor.matmul(ps, SGr_sb, eIm, start=False, stop=True)
                        nc.scalar.activation(C9[:, j * C + co:j * C + co + cs], ps,
                                             mybir.ActivationFunctionType.Copy)
                VA = act.tile([G, 9 * C], BF, tag="VA", name="VA")
                dv = VA.rearrange("(g j) (t c) -> g j t c", g=11, t=9)
                for j in range(9):
                    nc.sync.dma_start(out=dv[:, j], in_=C9[:, j * C:(j + 1) * C])
                return VA

            def s4_ln2_transpose(b, VA):
                """d-axis irfft (S4) + LN2 + PE transpose -> z0T [c, 891]."""
                z_tiles = []
                for t in range(9):
                    half = []
                    for co, cs in CCH:
                        ps = pspec.tile([G, 450], F32, tag="sp", name="psY")
                        nc.tensor.matmul(ps, A4_sb,
                                         VA[:, t * C + co:t * C + co + cs],
                                         start=True, stop=True)
                        half.append(ps)
                    st = stat.tile([128, 2, 6], F32, tag="st2")
                    nc.vector.bn_stats(st[:G, 0], half[0])
                    nc.vector.bn_stats(st[:G, 1], half[1])
                    mv = stat.tile([128, 2], F32, tag="mv2")
                    nc.vector.bn_aggr(mv[:G], st[:G])
                    rcp, nmu = ln_scalars(mv, G, "2")
                    z_t = act.tile([G, C], BF, tag=f"z{t}")
                    for ch, (co, cs) in enumerate(CCH):
                        nc.scalar.activation(z_t[:, co:co + cs], half[ch],
                                             mybir.ActivationFunctionType.Identity,
                                             bias=nmu[:G], scale=rcp[:G])
                    z_tiles.append(z_t)

                z0T = []
                for ct, (co, cs) in enumerate(C_T):
                    zt = act.tile([cs, N], BF, tag=f"zt{ct}")
                    for gi, grp in enumerate(((0, 1, 2, 3, 4), (5, 6, 7, 8))):
                        # inner dim padded to 100 so each block's PSUM byte
                        # offset stays 4-byte aligned (99*2 = 198 is not)
                        ps = pspec.tile([128, 5, G + 1], BF, tag="sp", name="ps_tp")
                        for ji, t in enumerate(grp):
                            nc.tensor.transpose(ps[:cs, ji, :G],
                                                z_tiles[t][:, co:co + cs],
                                                ident[:G, :G])
                        colo = 0 if gi == 0 else 5 * G
                        ncols = len(grp) * G
                        nc.vector.tensor_copy(
                            zt[:, colo:colo + ncols].rearrange(
                                "p (a b) -> p a b", b=G),
                            ps[:cs, 0:len(grp), 0:G])
                    z0T.append(zt)
                return z0T

            def mlp_l1(b, z0T):
                t1 = []
                for m, (mo, ms) in enumerate(R_T):
                    t_t = act.tile([ms, N], BF, tag=f"t1_{m}")
                    for nh, (no, ns) in enumerate(NH):
                        ps = psm.tile([128, 446], F32, tag="mm", name="psl1")
                        for kt, (ko, ks) in enumerate(C_T):
                            nc.tensor.matmul(ps[:ms, 0:ns],
                                             u1pT_sb[kt][:, mo:mo + ms],
                                             z0T[kt][:, no:no + ns],
                                             start=(kt == 0), stop=(kt == len(C_T) - 1))
                        nc.scalar.activation(t_t[:, no:no + ns], ps[:ms, 0:ns],
                                             mybir.ActivationFunctionType.Identity,
                                             bias=b1p_sb[:ms, m:m + 1], scale=1.0)
                    t1.append(t_t)
                return t1

            # ---- MLP tail, split into three chunks for pipelining ----
            def tail_alloc(t1):
                t3 = []
                for m, (mo, ms) in enumerate(R_T):
                    sz = ms + 1 if m == len(R_T) - 1 else ms
                    t3.append(act.tile([sz, N], BF, tag=f"t3_{m}", name=f"t3_{m}"))
                nc.sync.dma_start(out=t3[-1][RA_T[-1][1] - 1:RA_T[-1][1], :],
                                  in_=ones_d)
                return {"t1": t1, "t3": t3, "t2h": None}


            def tail_l2(state, nh):
                no, ns = NH[nh]
                t2h = []
                for m, (mo, ms) in enumerate(HID_T):
                    t_t = act.tile([ms, 446], BF, tag=f"t2_{m}")
                    ps = psm.tile([128, 446], F32, tag="mm")
                    for kt, (ko, ks) in enumerate(R_T):
                        nc.tensor.matmul(ps[:ms, 0:ns],
                                         v1T_sb[kt][:, mo:mo + ms],
                                         state["t1"][kt][:, no:no + ns],
                                         start=(kt == 0), stop=(kt == len(R_T) - 1))
                    nc.scalar.activation(t_t[:, 0:ns], ps[:ms, 0:ns],
                                         gelu_func,
                                         bias=v1bp_sb[:ms, m:m + 1], scale=1.0)
                    t2h.append(t_t)
                state["t2h"] = t2h

            def tail_l3(state, nh):
                no, ns = NH[nh]
                t2h, t3 = state["t2h"], state["t3"]
                for m, (mo, ms) in enumerate(R_T):
                    ps = psm.tile([128, 446], F32, tag="mm")
                    for kt, (ko, ks) in enumerate(HID_T):
                        nc.tensor.matmul(ps[:ms, 0:ns],
                                         u2T_sb[kt][:, mo:mo + ms],
                                         t2h[kt][:, 0:ns],
                                         start=(kt == 0), stop=(kt == len(HID_T) - 1))
                    nc.vector.tensor_copy(t3[m][:ms, no:no + ns], ps[:ms, 0:ns])

            def tail_l4(b, state):
                t3 = state["t3"]
                for rt, (ro, rs) in enumerate(ROW_T):
                    xr = xres.tile([128, C], F32, tag="xr")
                    nc.sync.dma_start(out=xr[:rs], in_=x_d[b, ro:ro + rs, :])
                    for ch, (co, cs) in enumerate(CCH):
                        ps = psm.tile([128, 450], F32, tag="mm", name="psl4")
                        for kt, (ko, ks) in enumerate(RA_T):
                            nc.tensor.matmul(ps[:rs, 0:cs],
                                             t3[kt][:, ro:ro + rs],
                                             v2Ta_sb[kt][:, co:co + cs],
                                             start=(kt == 0), stop=(kt == len(RA_T) - 1))
                        nc.vector.tensor_add(xr[:rs, co:co + cs], xr[:rs, co:co + cs],
                                             ps[:rs, 0:cs])
                    nc.sync.dma_start(out=out_d[b, ro:ro + rs, :], in_=xr[:rs])

            # software pipeline: batch b-1's MLP tail chunks are emitted
            # between batch b's spectral phases so PE always has dense work
            # while DMA regroups / DVE cmult / LN chains run.
            pending = None
            for b in range(bl):
                Dt = ln1_s1_fwd(b)
                if b == 0:
                    load_mlp_consts()
                if pending is not None:
                    tail_l2(pending[1], 0)
                Et = s2_cmult(b, Dt)
                if pending is not None:
                    tail_l3(pending[1], 0)
                    tail_l2(pending[1], 1)
                VA = s3_regroup(b, Et)
                if pending is not None:
                    tail_l3(pending[1], 1)
                    tail_l4(pending[0], pending[1])
                z0T = s4_ln2_transpose(b, VA)
                t1 = mlp_l1(b, z0T)
                pending = (b, tail_alloc(t1))
            tail_l2(pending[1], 0)
            tail_l3(pending[1], 0)
            tail_l2(pending[1], 1)
            tail_l3(pending[1], 1)
            tail_l4(pending[0], pending[1])

    nc.compile()
    return nc


_CACHE = {}


def kernel(**inputs):
    if "nc" not in _CACHE:
        _CACHE["nc"] = build_module(BL)
    nc = _CACHE["nc"]
    consts = _host_constants(inputs)
    x = np.ascontiguousarray(np.asarray(inputs["x"], np.float32))
    in_maps = []
    for c in range(NCORES):
        m = {"x": np.ascontiguousarray(x[c * BL:(c + 1) * BL])}
        m.update(consts)
        in_maps.append(m)
    res = run_bass_kernel_spmd(nc, in_maps, core_ids=list(range(NCORES)))
    out = np.concatenate([r["out"] for r in res.results], axis=0)
    return out.astype(np.float32)
